# revision 12
# baseline (speedup 1.0000x reference)
"""Causal multi-head attention layer on 8 Trainium2 NeuronCores.

Sharding: tensor-parallel over heads (16 heads -> 2 per core).
fp16 data path (x, W, q/k/v, att, attout, W_out), f32 PSUM accumulation.

Per core, for its 2 heads:
  qkv^T = W_slice^T @ x^T             (fp16 matmuls, x pre-transposed on host)
  S^T[k,q] = K^T_chunk^T @ Q^T        (scores transposed; 2 heads row-packed
                                       in PE via tile_position=(h*64, 0))
  att^T = exp(S^T/8) fp16             (h-merged ACT instr per key chunk;
                                       triangular mask on diagonal blocks)
  denom = partition_all_reduce(sum_kc att)   (DVE accum + gpsimd reduce)
  out^T[dv,q] = V^T-stationary @ att^T (2 heads col-packed: h0 -> PE cols
                                       0:64 / PSUM parts 0:64, h1 -> 64:128)
  attout^T = out^T * recip(denom)
  partial^T[e,tok] = W_out_slice chunks @ attout^T -> DMA direct from PSUM
Host: sum partials over cores, transpose, + b_out.
"""
import numpy as np

import concourse.bacc as bacc
import concourse.bass as bass
import concourse.bass_isa as bass_isa
import concourse.mybir as mybir
import concourse.tile as tile
from concourse import bass_utils

B, S, E, H = 4, 2048, 1024, 16
D = E // H            # 64
TOK = B * S           # 8192
KC = E // 128         # 8 emb chunks
TB = 512              # qkv token block
QB = 1024             # attention q block
NB = S // TB          # 4 token blocks per batch
NQB = S // QB         # 2 q blocks per batch

f32 = mybir.dt.float32
f16 = mybir.dt.float16
f8 = mybir.dt.float8e4
FT = mybir.ActivationFunctionType
MUL = mybir.AluOpType.mult
ADD = mybir.AluOpType.add
DR = mybir.MatmulPerfMode.DoubleRow


def splits(lo, hi, step=512):
    """Split [lo, hi) into pieces aligned to `step` boundaries."""
    out = []
    p = lo
    while p < hi:
        q = min((p // step + 1) * step, hi)
        out.append((p, q))
        p = q
    return out


def build(repeats: int = 1):
    nc = bacc.Bacc("TRN2", target_bir_lowering=False, debug=False, num_devices=8)
    xT = nc.dram_tensor("xT", [E, TOK], f16, kind="ExternalInput")
    xT8 = nc.dram_tensor("xT8", [E, TOK], f8, kind="ExternalInput")
    wq = nc.dram_tensor("wq", [E, 128], f8, kind="ExternalInput")
    wk = nc.dram_tensor("wk", [E, 128], f8, kind="ExternalInput")
    wv = nc.dram_tensor("wv", [E, 128], f16, kind="ExternalInput")
    wo = nc.dram_tensor("wo", [128, E], f16, kind="ExternalInput")
    bq = nc.dram_tensor("bq", [128, 1], f32, kind="ExternalInput")
    bk = nc.dram_tensor("bk", [128, 1], f32, kind="ExternalInput")
    bv = nc.dram_tensor("bv", [128, 1], f32, kind="ExternalInput")
    tri = nc.dram_tensor("tri", [128, 128], f16, kind="ExternalInput")
    idd = nc.dram_tensor("idd", [128, 128], f16, kind="ExternalInput")
    outp = nc.dram_tensor("outp", [E, TOK], f16, kind="ExternalOutput")

    with tile.TileContext(nc) as tc:
        with (
            tc.tile_pool(name="wp", bufs=1) as wp,
            tc.tile_pool(name="xp", bufs=2) as xp,
            tc.tile_pool(name="qk", bufs=2) as qk,
            tc.tile_pool(name="at", bufs=1) as atp,
            tc.tile_pool(name="ac", bufs=2) as acp,
            tc.tile_pool(name="dn", bufs=2) as dnp,
            tc.tile_pool(name="ao", bufs=2) as aop,
            tc.tile_pool(name="po", bufs=3) as pop,
            tc.tile_pool(name="psS", bufs=1, space="PSUM") as psS,
            tc.tile_pool(name="psA", bufs=2, space="PSUM") as psA,
            tc.tile_pool(name="psO", bufs=2, space="PSUM") as psO,
        ):
            # --- constants / weights (loaded once) ---
            # first QKV token block's x slice loads FIRST so the PE can
            # start as soon as wq lands; remaining weights follow.
            xt00 = ([], [])
            for hf in range(2):
                x8 = xp.tile([128, KC * TB // 2], f8, tag=f"x8{hf}",
                             name=f"x8_pre0_{hf}")
                nc.sync.dma_start(
                    x8[:].rearrange("p (c m) -> p c m", c=KC // 2),
                    xT8.ap()[hf * (E // 2):(hf + 1) * (E // 2),
                             0:TB].rearrange("(c p) m -> p c m", p=128),
                )
                xt00[1].append(x8)
            for hf in range(2):
                x1 = xp.tile([128, KC * TB // 2], f16, tag=f"xt{hf}",
                             name=f"xt_pre0_{hf}")
                nc.sync.dma_start(
                    x1[:].rearrange("p (c m) -> p c m", c=KC // 2),
                    xT.ap()[hf * (E // 2):(hf + 1) * (E // 2), 0:TB].rearrange(
                        "(c p) m -> p c m", p=128),
                )
                xt00[0].append(x1)
            wq_sb = wp.tile([128, E], f8)
            wk_sb = wp.tile([128, E], f8)
            wv_sb = wp.tile([128, E], f16)
            wo_sb = wp.tile([128, E], f16)
            bq_sb = wp.tile([128, 1], f32)
            bk_sb = wp.tile([128, 1], f32)
            bv_sb = wp.tile([128, 1], f32)
            for hf in range(2):
                nc.sync.dma_start(
                    wq_sb[:, hf * (E // 2):(hf + 1) * (E // 2)].rearrange(
                        "p (c m) -> p c m", c=KC // 2),
                    wq.ap()[hf * (E // 2):(hf + 1) * (E // 2), :].rearrange(
                        "(c p) m -> p c m", p=128),
                )
            nc.sync.dma_start(bq_sb[:], bq.ap())
            for wsb_, wdr_ in ((wk_sb, wk), (wv_sb, wv)):
                nc.sync.dma_start(
                    wsb_[:].rearrange("p (c m) -> p c m", c=KC),
                    wdr_.ap().rearrange("(c p) m -> p c m", p=128),
                )
            nc.sync.dma_start(wo_sb[:], wo.ap())
            nc.sync.dma_start(bk_sb[:], bk.ap())
            nc.sync.dma_start(bv_sb[:], bv.ap())
            tri_sb = wp.tile([128, 128], f16)
            nc.sync.dma_start(tri_sb[:], tri.ap())
            id_sb = wp.tile([128, 128], f16)
            nc.sync.dma_start(id_sb[:], idd.ap())
            # preload ACT exp table set during the prologue
            warm = wp.tile([1, 1], f32)
            nc.vector.memset(warm[:], 0.0)
            nc.scalar.activation(warm[:], warm[:], FT.Exp, scale=1.0)
            # persistent V-transposed tiles [128 keys, 128 = 2 heads x 64 dv]
            vns = [
                wp.tile([128, 128], f16, tag=f"vn{i}", name=f"vn{i}")
                for i in range(S // 128)
            ]

            def alloc_qkv(b):
                return (
                    qk.tile([128, S], f16, tag="qT", name=f"qT{b}"),
                    qk.tile([128, S], f16, tag="kT", name=f"kT{b}"),
                    qk.tile([128, S], f16, tag="vT", name=f"vT{b}"),
                )

            def qkv_dma(b, t, rep):
                tok0 = b * S + t * TB
                xth, xth8 = [], []
                for hf in range(2):
                    x8 = xp.tile([128, KC * TB // 2], f8, tag=f"x8{hf}",
                                 name=f"x8{rep}_{b}_{t}_{hf}")
                    nc.sync.dma_start(
                        x8[:].rearrange("p (c m) -> p c m", c=KC // 2),
                        xT8.ap()[hf * (E // 2):(hf + 1) * (E // 2),
                                 tok0:tok0 + TB].rearrange(
                            "(c p) m -> p c m", p=128),
                    )
                    xth8.append(x8)
                for hf in range(2):
                    x1 = xp.tile([128, KC * TB // 2], f16, tag=f"xt{hf}",
                                 name=f"xt{rep}_{b}_{t}_{hf}")
                    nc.sync.dma_start(
                        x1[:].rearrange("p (c m) -> p c m", c=KC // 2),
                        xT.ap()[hf * (E // 2):(hf + 1) * (E // 2),
                                tok0:tok0 + TB].rearrange(
                            "(c p) m -> p c m", p=128),
                    )
                    xth.append(x1)
                return (xth, xth8)

            def qkv_group(b, t, tiles, xpair, gi, rep):
                qT, kT, vT = tiles
                xth, xth8 = xpair
                wsb, bsb, dst = (
                    (wq_sb, bq_sb, qT), (wk_sb, bk_sb, kT),
                    (wv_sb, bv_sb, vT),
                )[gi]
                ps = psA.tile([128, TB], f32, tag="mm512",
                              name=f"psqkv{rep}_{b}_{t}_{gi}")
                if gi < 2:
                    # Q/K projections in fp8 DoubleRow: each matmul contracts
                    # 256 rows (2 k-tiles packed per PE cell)
                    for c in range(KC // 2):
                        x8 = xth8[c // 2]
                        cl = 2 * (c % 2)
                        nc.tensor.matmul(
                            ps[:],
                            wsb[:, (2 * c) * 128:(2 * c + 2) * 128].rearrange(
                                "p (i m) -> p i m", i=2),
                            x8[:, cl * TB:(cl + 2) * TB].rearrange(
                                "p (i m) -> p i m", i=2),
                            start=(c == 0), stop=(c == KC // 2 - 1),
                            perf_mode=DR,
                        )
                else:
                    for kc in range(KC):
                        xsrc = xth[kc // (KC // 2)]
                        nc.tensor.matmul(
                            ps[:],
                            wsb[:, kc * 128:(kc + 1) * 128],
                            xsrc[:, (kc % (KC // 2)) * TB:
                                 (kc % (KC // 2) + 1) * TB],
                            start=(kc == 0), stop=(kc == KC - 1),
                        )
                nc.vector.tensor_scalar_add(
                    dst[:, t * TB:(t + 1) * TB], ps[:], bsb[:]
                )

            def vnat(b, tiles, rep, lo=0, hi=S // 128):
                vT = tiles[2]
                for i in range(lo, hi):
                    pst = psA.tile([128, 128], f16, tag="mm512",
                                   name=f"pst{rep}_{b}_{i}")
                    nc.tensor.transpose(
                        pst[:], vT[:, i * 128:(i + 1) * 128], id_sb[:]
                    )
                    nc.vector.tensor_copy(vns[i][:], pst[:])

            def scores(b, qb, tiles, att, acc, rep, fill=()):
                qT, kT, vT = tiles
                q0 = qb * QB
                nkc = (q0 + QB) // 128
                fill = list(fill)
                nf = len(fill)
                fired = 0
                for kc in range(nkc):
                    kst = kc * 128
                    r0 = max(0, kst - q0)
                    off = kc * 2 * QB
                    ps_s = psS.tile([128, 2 * QB], f32, tag="s",
                                    name=f"pss{rep}_{b}_{qb}_{kc}")
                    for h in range(2):
                        hs = slice(h * 64, (h + 1) * 64)
                        for (p0, p1) in splits(r0, QB):
                            nc.tensor.matmul(
                                ps_s[:, h * QB + p0:h * QB + p1],
                                kT[hs, kst:kst + 128],
                                qT[hs, q0 + p0:q0 + p1],
                                start=True, stop=True,
                                tile_position=(h * 64, 0),
                            )
                    if r0 == 0:
                        nc.scalar.activation(
                            att[:, off:off + 2 * QB],
                            ps_s[:, 0:2 * QB],
                            FT.Exp, scale=0.125,
                        )
                    else:
                        for h in range(2):
                            nc.scalar.activation(
                                att[:, off + h * QB + r0:off + (h + 1) * QB],
                                ps_s[:, h * QB + r0:(h + 1) * QB],
                                FT.Exp, scale=0.125,
                            )
                    if kst >= q0:
                        for h in range(2):
                            blk = att[:, off + h * QB + r0:
                                      off + h * QB + r0 + 128]
                            nc.vector.tensor_tensor(blk, blk, tri_sb[:],
                                                    op=MUL)
                    # accumulate per-key-chunk sums for the softmax denom
                    if kc == 0:
                        nc.vector.tensor_copy(
                            acc[:, 0:2 * QB], att[:, off:off + 2 * QB])
                    elif r0 == 0:
                        # SBUF-only adds are legal on Pool; offload every
                        # third one to keep DVE under the PE roofline
                        eng = nc.gpsimd if kc % 3 == 2 else nc.vector
                        eng.tensor_tensor(
                            acc[:, 0:2 * QB], acc[:, 0:2 * QB],
                            att[:, off:off + 2 * QB], op=ADD)
                    else:
                        for h in range(2):
                            sa = slice(h * QB + r0, (h + 1) * QB)
                            st = slice(off + h * QB + r0, off + (h + 1) * QB)
                            nc.vector.tensor_tensor(
                                acc[:, sa], acc[:, sa], att[:, st], op=ADD)
                    # interleave PE fill work (next batch QKV, outproj) to
                    # cover the ACT exp-throughput deficit
                    want = (kc + 1) * nf // nkc
                    while fired < want:
                        fill[fired]()
                        fired += 1
                while fired < nf:
                    fill[fired]()
                    fired += 1

            def denom(b, qb, acc, rep):
                dn = dnp.tile([128, 2 * QB], f32, tag="dn",
                              name=f"dn{rep}_{b}_{qb}")
                nc.gpsimd.partition_all_reduce(
                    dn[:], acc[:], channels=128,
                    reduce_op=bass_isa.ReduceOp.add,
                )
                rcp = dnp.tile([128, 2 * QB], f16, tag="rcp",
                               name=f"rcp{rep}_{b}_{qb}")
                with nc.allow_low_precision(reason="softmax recip in fp16"):
                    nc.vector.reciprocal(rcp[:], dn[:])
                return rcp

            def attv_qbb(b, qb, qbb, att, rcp, aos, rep):
                q0 = qb * QB
                qa0 = q0 + qbb * 512
                nkc_q = (qa0 + 512) // 128
                ps_o = psO.tile([128, 512], f32, tag="o",
                                name=f"pso{rep}_{b}_{qb}_{qbb}")
                for kc in range(nkc_q):
                    kst = kc * 128
                    lo = max(qa0, kst) - qa0
                    off = kc * 2 * QB
                    vn = vns[kc]
                    for h in range(2):
                        nc.tensor.matmul(
                            ps_o[h * 64:(h + 1) * 64, lo:512],
                            vn[:, h * 64:(h + 1) * 64],
                            att[:, off + h * QB + qbb * 512 + lo:
                                off + h * QB + (qbb + 1) * 512],
                            start=(kc == 0), stop=(kc == nkc_q - 1),
                            tile_position=(0, h * 64),
                            skip_group_check=True,
                        )
                for h in range(2):
                    nc.vector.tensor_tensor(
                        aos[h * 64:(h + 1) * 64, qa0:qa0 + 512],
                        ps_o[h * 64:(h + 1) * 64, :],
                        rcp[h * 64:(h + 1) * 64,
                            h * QB + qbb * 512:h * QB + (qbb + 1) * 512],
                        op=MUL,
                    )

            def outproj_ec(b, half, ec, aos, rep):
                t0b = b * S
                po = pop.tile([128, 2 * TB], f16, tag="po",
                              name=f"po{rep}_{b}_{ec}_{half}")
                for tt in range(NB // 2):
                    t = half * (NB // 2) + tt
                    ps_p = psA.tile([128, TB], f32, tag="mm512",
                                    name=f"psp{rep}_{b}_{ec}_{t}")
                    nc.tensor.matmul(
                        ps_p[:],
                        wo_sb[:, ec * 128:(ec + 1) * 128],
                        aos[:, t * TB:(t + 1) * TB],
                        start=True, stop=True,
                    )
                    # PSUM is only reachable from DVE/ACT; put 1/4 of the
                    # drains on ScalarE, the rest on DVE
                    dst = po[:, tt * TB:(tt + 1) * TB]
                    if (ec * 2 + tt) % 4 == 3:
                        nc.scalar.copy(dst, ps_p[:])
                    else:
                        nc.vector.tensor_copy(dst, ps_p[:])
                nc.sync.dma_start(
                    outp.ap()[ec * 128:(ec + 1) * 128,
                              t0b + half * (S // 2):
                              t0b + (half + 1) * (S // 2)],
                    po[:],
                )

            def outproj_half(b, half, aos, rep):
                for ec in range(KC):
                    outproj_ec(b, half, ec, aos, rep)

            for rep in range(repeats):
                # prologue: only batch-0 tokens [0,1024) serially; the rest
                # becomes fill work inside the first scores loop
                tiles = alloc_qkv(0)
                for t in (0, 1):
                    xth = xt00 if (t == 0 and rep == 0) else qkv_dma(0, t, rep)
                    for gi in range(3):
                        qkv_group(0, t, tiles, xth, gi, rep)
                vnat(0, tiles, rep, 0, 8)
                pro_fill = []
                for t in (2, 3):
                    xth = qkv_dma(0, t, rep)
                    for gi in range(3):
                        pro_fill.append(
                            (lambda t=t, xth=xth, gi=gi, tl=tiles:
                             qkv_group(0, t, tl, xth, gi, rep))
                        )
                pro_fill.append(
                    (lambda tl=tiles: vnat(0, tl, rep, 8, S // 128))
                )
                prev = None  # (b, aos) with half-1 outproj still pending
                for b in range(B):
                    nxt = b + 1 if b + 1 < B else None
                    tiles_next = alloc_qkv(nxt) if nxt is not None else None
                    aos = aop.tile([128, S], f16, tag="ao",
                                   name=f"ao{rep}_{b}")
                    for qb in range(NQB):
                        att = atp.tile([128, (8 if qb == 0 else 16) * 2 * QB],
                                       f16, tag=f"att{qb}",
                                       name=f"att{rep}_{b}_{qb}")
                        acc = acp.tile([128, 2 * QB], f16, tag="acc",
                                       name=f"acc{rep}_{b}_{qb}")
                        fill = []
                        if b == 0 and qb == 0:
                            fill.extend(pro_fill)
                        if nxt is not None:
                            tls = [0] if qb == 0 else [1, 2, 3]
                            for t in tls:
                                xth = qkv_dma(nxt, t, rep)
                                for gi in range(3):
                                    fill.append(
                                        (lambda t=t, xth=xth, gi=gi:
                                         qkv_group(nxt, t, tiles_next,
                                                   xth, gi, rep))
                                    )
                        if qb == 0 and prev is not None:
                            pb, paos = prev
                            for ec in range(KC):
                                fill.append(
                                    (lambda ec=ec, pb=pb, paos=paos:
                                     outproj_ec(pb, 1, ec, paos, rep))
                                )
                            prev = None
                        if qb == 1:
                            for ec in range(KC):
                                fill.append(
                                    (lambda ec=ec: outproj_ec(b, 0, ec,
                                                              aos, rep))
                                )
                        scores(b, qb, tiles, att, acc, rep, fill)
                        rcp = denom(b, qb, acc, rep)
                        for qbb in range(QB // 512):
                            attv_qbb(b, qb, qbb, att, rcp, aos, rep)
                    if nxt is not None:
                        vnat(nxt, tiles_next, rep)
                    prev = (b, aos)
                    tiles = tiles_next
                pb, paos = prev
                outproj_half(pb, 1, paos, rep)
    nc.compile()
    return nc


_CACHE = {}


def _get_nc(repeats=1):
    if repeats not in _CACHE:
        _CACHE[repeats] = build(repeats)
    return _CACHE[repeats]


def make_in_maps(x, W_qkv, b_qkv, W_out, b_out):
    x = np.asarray(x, dtype=np.float32)
    W_qkv = np.asarray(W_qkv, dtype=np.float32)
    b_qkv = np.asarray(b_qkv, dtype=np.float32)
    W_out = np.asarray(W_out, dtype=np.float32)
    np8 = mybir.dt.np(f8)
    xT = np.ascontiguousarray(x.reshape(TOK, E).T).astype(np.float16)
    xT8 = xT.astype(np8)
    trim = np.ascontiguousarray(
        np.triu(np.ones((128, 128), dtype=np.float32))
    ).astype(np.float16)
    in_maps = []
    for c in range(8):
        cs = slice(c * 128, (c + 1) * 128)
        in_maps.append({
            "xT": xT,
            "xT8": xT8,
            "wq": np.ascontiguousarray(
                W_qkv[:, c * 128:(c + 1) * 128]).astype(np8),
            "wk": np.ascontiguousarray(
                W_qkv[:, E + c * 128:E + (c + 1) * 128]).astype(np8),
            "wv": np.ascontiguousarray(
                W_qkv[:, 2 * E + c * 128:2 * E + (c + 1) * 128]
            ).astype(np.float16),
            "wo": np.ascontiguousarray(W_out[cs, :]).astype(np.float16),
            "bq": np.ascontiguousarray(b_qkv[c * 128:(c + 1) * 128, None]),
            "bk": np.ascontiguousarray(
                b_qkv[E + c * 128:E + (c + 1) * 128, None]),
            "bv": np.ascontiguousarray(
                b_qkv[2 * E + c * 128:2 * E + (c + 1) * 128, None]),
            "tri": trim,
            "idd": np.eye(128, dtype=np.float32).astype(np.float16),
        })
    return in_maps


def gather(results, b_out):
    total = np.zeros((E, TOK), dtype=np.float64)
    for c in range(8):
        total += results[c]["outp"].astype(np.float64)
    out = total.T.astype(np.float32) + np.asarray(b_out, dtype=np.float32)
    return np.ascontiguousarray(out.reshape(B, S, E)).astype(np.float32)


def kernel(x, W_qkv, b_qkv, W_out, b_out):
    nc = _get_nc(1)
    in_maps = make_in_maps(x, W_qkv, b_qkv, W_out, b_out)
    res = bass_utils.run_bass_kernel_spmd(nc, in_maps, core_ids=list(range(8)))
    return gather(res.results, b_out)


# revision 18
# speedup vs baseline: 1.4084x; 1.4084x over previous
"""Causal multi-head attention layer on 8 Trainium2 NeuronCores.

Sharding: tensor-parallel over heads (16 heads -> 2 per core).
fp16 data path (x, W, q/k/v, att, attout, W_out), f32 PSUM accumulation.

Per core, for its 2 heads:
  qkv^T = W_slice^T @ x^T             (fp16 matmuls, x pre-transposed on host)
  S^T[k,q] = K^T_chunk^T @ Q^T        (scores transposed; 2 heads row-packed
                                       in PE via tile_position=(h*64, 0))
  att^T = exp(S^T/8) fp16             (h-merged ACT instr per key chunk;
                                       triangular mask on diagonal blocks)
  denom = partition_all_reduce(sum_kc att)   (DVE accum + gpsimd reduce)
  out^T[dv,q] = V^T-stationary @ att^T (2 heads col-packed: h0 -> PE cols
                                       0:64 / PSUM parts 0:64, h1 -> 64:128)
  attout^T = out^T * recip(denom)
  partial^T[e,tok] = W_out_slice chunks @ attout^T -> DMA direct from PSUM
Host: sum partials over cores, transpose, + b_out.
"""
import numpy as np

import concourse.bacc as bacc
import concourse.bass as bass
import concourse.bass_isa as bass_isa
import concourse.mybir as mybir
import concourse.tile as tile
from concourse import bass_utils

B, S, E, H = 4, 2048, 1024, 16
D = E // H            # 64
TOK = B * S           # 8192
KC = E // 128         # 8 emb chunks
TB = 512              # qkv token block
QB = 1024             # attention q block
NB = S // TB          # 4 token blocks per batch
NQB = S // QB         # 2 q blocks per batch

f32 = mybir.dt.float32
f16 = mybir.dt.float16
f8 = mybir.dt.float8e4
FT = mybir.ActivationFunctionType
MUL = mybir.AluOpType.mult
ADD = mybir.AluOpType.add
DR = mybir.MatmulPerfMode.DoubleRow


def splits(lo, hi, step=512):
    """Split [lo, hi) into pieces aligned to `step` boundaries."""
    out = []
    p = lo
    while p < hi:
        q = min((p // step + 1) * step, hi)
        out.append((p, q))
        p = q
    return out


def build(repeats: int = 1):
    nc = bacc.Bacc("TRN2", target_bir_lowering=False, debug=False, num_devices=8)
    xT = nc.dram_tensor("xT", [E, TOK], f16, kind="ExternalInput")
    wq = nc.dram_tensor("wq", [E, 128], f16, kind="ExternalInput")
    wk = nc.dram_tensor("wk", [E, 128], f16, kind="ExternalInput")
    wv = nc.dram_tensor("wv", [E, 128], f16, kind="ExternalInput")
    wo = nc.dram_tensor("wo", [128, E], f16, kind="ExternalInput")
    bq = nc.dram_tensor("bq", [128, 1], f32, kind="ExternalInput")
    bk = nc.dram_tensor("bk", [128, 1], f32, kind="ExternalInput")
    bv = nc.dram_tensor("bv", [128, 1], f32, kind="ExternalInput")
    tri = nc.dram_tensor("tri", [128, 128], f16, kind="ExternalInput")
    idd = nc.dram_tensor("idd", [128, 128], f16, kind="ExternalInput")
    outp = nc.dram_tensor("outp", [E, TOK], f16, kind="ExternalOutput")

    with tile.TileContext(nc) as tc:
        with (
            tc.tile_pool(name="wp", bufs=1) as wp,
            tc.tile_pool(name="xp", bufs=2) as xp,
            tc.tile_pool(name="qk", bufs=2) as qk,
            tc.tile_pool(name="at", bufs=1) as atp,
            tc.tile_pool(name="ac", bufs=2) as acp,
            tc.tile_pool(name="dn", bufs=2) as dnp,
            tc.tile_pool(name="ao", bufs=2) as aop,
            tc.tile_pool(name="po", bufs=3) as pop,
            tc.tile_pool(name="psS", bufs=1, space="PSUM") as psS,
            tc.tile_pool(name="psA", bufs=2, space="PSUM") as psA,
            tc.tile_pool(name="psO", bufs=2, space="PSUM") as psO,
        ):
            # --- constants / weights (loaded once) ---
            # first QKV token block's x slice loads FIRST so the PE can
            # start as soon as wq lands; remaining weights follow.
            xt00 = []
            for hf in range(2):
                x1 = xp.tile([128, KC * TB // 2], f16, tag=f"xt{hf}",
                             name=f"xt_pre0_{hf}")
                nc.sync.dma_start(
                    x1[:].rearrange("p (c m) -> p c m", c=KC // 2),
                    xT.ap()[hf * (E // 2):(hf + 1) * (E // 2), 0:TB].rearrange(
                        "(c p) m -> p c m", p=128),
                )
                xt00.append(x1)
            wq_sb = wp.tile([128, E], f16)
            wk_sb = wp.tile([128, E], f16)
            wv_sb = wp.tile([128, E], f16)
            wo_sb = wp.tile([128, E], f16)
            bq_sb = wp.tile([128, 1], f32)
            bk_sb = wp.tile([128, 1], f32)
            bv_sb = wp.tile([128, 1], f32)
            for hf in range(2):
                nc.sync.dma_start(
                    wq_sb[:, hf * (E // 2):(hf + 1) * (E // 2)].rearrange(
                        "p (c m) -> p c m", c=KC // 2),
                    wq.ap()[hf * (E // 2):(hf + 1) * (E // 2), :].rearrange(
                        "(c p) m -> p c m", p=128),
                )
            nc.sync.dma_start(bq_sb[:], bq.ap())
            for wsb_, wdr_ in ((wk_sb, wk), (wv_sb, wv)):
                nc.sync.dma_start(
                    wsb_[:].rearrange("p (c m) -> p c m", c=KC),
                    wdr_.ap().rearrange("(c p) m -> p c m", p=128),
                )
            nc.sync.dma_start(wo_sb[:], wo.ap())
            nc.sync.dma_start(bk_sb[:], bk.ap())
            nc.sync.dma_start(bv_sb[:], bv.ap())
            tri_sb = wp.tile([128, 128], f16)
            nc.sync.dma_start(tri_sb[:], tri.ap())
            id_sb = wp.tile([128, 128], f16)
            nc.sync.dma_start(id_sb[:], idd.ap())
            # preload ACT exp table set during the prologue
            warm = wp.tile([1, 1], f32)
            nc.vector.memset(warm[:], 0.0)
            nc.scalar.activation(warm[:], warm[:], FT.Exp, scale=1.0)
            # persistent V-transposed tiles [128 keys, 128 = 2 heads x 64 dv]
            vns = [
                wp.tile([128, 128], f16, tag=f"vn{i}", name=f"vn{i}")
                for i in range(S // 128)
            ]

            def alloc_qkv(b):
                return (
                    qk.tile([128, S], f16, tag="qT", name=f"qT{b}"),
                    qk.tile([128, S], f16, tag="kT", name=f"kT{b}"),
                    qk.tile([128, S], f16, tag="vT", name=f"vT{b}"),
                )

            def qkv_dma(b, t, rep):
                tok0 = b * S + t * TB
                xth = []
                for hf in range(2):
                    x1 = xp.tile([128, KC * TB // 2], f16, tag=f"xt{hf}",
                                 name=f"xt{rep}_{b}_{t}_{hf}")
                    nc.sync.dma_start(
                        x1[:].rearrange("p (c m) -> p c m", c=KC // 2),
                        xT.ap()[hf * (E // 2):(hf + 1) * (E // 2),
                                tok0:tok0 + TB].rearrange(
                            "(c p) m -> p c m", p=128),
                    )
                    xth.append(x1)
                return xth

            def qkv_group(b, t, tiles, xth, gi, rep):
                qT, kT, vT = tiles
                wsb, bsb, dst = (
                    (wq_sb, bq_sb, qT), (wk_sb, bk_sb, kT),
                    (wv_sb, bv_sb, vT),
                )[gi]
                ps = psA.tile([128, TB], f32, tag="mm512",
                              name=f"psqkv{rep}_{b}_{t}_{gi}")
                for kc in range(KC):
                    xsrc = xth[kc // (KC // 2)]
                    nc.tensor.matmul(
                        ps[:],
                        wsb[:, kc * 128:(kc + 1) * 128],
                        xsrc[:, (kc % (KC // 2)) * TB:
                             (kc % (KC // 2) + 1) * TB],
                        start=(kc == 0), stop=(kc == KC - 1),
                    )
                nc.vector.tensor_scalar_add(
                    dst[:, t * TB:(t + 1) * TB], ps[:], bsb[:]
                )

            def vnat(b, tiles, rep, lo=0, hi=S // 128):
                vT = tiles[2]
                for i in range(lo, hi):
                    pst = psA.tile([128, 128], f16, tag="mm512",
                                   name=f"pst{rep}_{b}_{i}")
                    nc.tensor.transpose(
                        pst[:], vT[:, i * 128:(i + 1) * 128], id_sb[:]
                    )
                    nc.vector.tensor_copy(vns[i][:], pst[:])

            def scores(b, qb, tiles, att, acc, rep, fill=()):
                qT, kT, vT = tiles
                q0 = qb * QB
                nkc = (q0 + QB) // 128
                fill = list(fill)
                nf = len(fill)
                fired = 0
                for kc in range(nkc):
                    kst = kc * 128
                    r0 = max(0, kst - q0)
                    off = kc * 2 * QB
                    ps_s = psS.tile([128, 2 * QB], f32, tag="s",
                                    name=f"pss{rep}_{b}_{qb}_{kc}")
                    for h in range(2):
                        hs = slice(h * 64, (h + 1) * 64)
                        for (p0, p1) in splits(r0, QB):
                            nc.tensor.matmul(
                                ps_s[:, h * QB + p0:h * QB + p1],
                                kT[hs, kst:kst + 128],
                                qT[hs, q0 + p0:q0 + p1],
                                start=True, stop=True,
                                tile_position=(h * 64, 0),
                            )
                    if r0 == 0:
                        nc.scalar.activation(
                            att[:, off:off + 2 * QB],
                            ps_s[:, 0:2 * QB],
                            FT.Exp, scale=0.125,
                        )
                    else:
                        for h in range(2):
                            nc.scalar.activation(
                                att[:, off + h * QB + r0:off + (h + 1) * QB],
                                ps_s[:, h * QB + r0:(h + 1) * QB],
                                FT.Exp, scale=0.125,
                            )
                    if kst >= q0:
                        for h in range(2):
                            blk = att[:, off + h * QB + r0:
                                      off + h * QB + r0 + 128]
                            nc.vector.tensor_tensor(blk, blk, tri_sb[:],
                                                    op=MUL)
                    # accumulate per-key-chunk sums for the softmax denom
                    if kc == 0:
                        nc.vector.tensor_copy(
                            acc[:, 0:2 * QB], att[:, off:off + 2 * QB])
                    elif r0 == 0:
                        # SBUF-only adds are legal on Pool; offload every
                        # third one to keep DVE under the PE roofline
                        eng = nc.gpsimd if kc % 3 == 2 else nc.vector
                        eng.tensor_tensor(
                            acc[:, 0:2 * QB], acc[:, 0:2 * QB],
                            att[:, off:off + 2 * QB], op=ADD)
                    else:
                        for h in range(2):
                            sa = slice(h * QB + r0, (h + 1) * QB)
                            st = slice(off + h * QB + r0, off + (h + 1) * QB)
                            nc.vector.tensor_tensor(
                                acc[:, sa], acc[:, sa], att[:, st], op=ADD)
                    # interleave PE fill work (next batch QKV, outproj) to
                    # cover the ACT exp-throughput deficit
                    want = (kc + 1) * nf // nkc
                    while fired < want:
                        fill[fired]()
                        fired += 1
                while fired < nf:
                    fill[fired]()
                    fired += 1

            def denom(b, qb, acc, rep):
                dn = dnp.tile([128, 2 * QB], f32, tag="dn",
                              name=f"dn{rep}_{b}_{qb}")
                nc.gpsimd.partition_all_reduce(
                    dn[:], acc[:], channels=128,
                    reduce_op=bass_isa.ReduceOp.add,
                )
                rcp = dnp.tile([128, 2 * QB], f16, tag="rcp",
                               name=f"rcp{rep}_{b}_{qb}")
                with nc.allow_low_precision(reason="softmax recip in fp16"):
                    nc.vector.reciprocal(rcp[:], dn[:])
                return rcp

            def attv_qbb(b, qb, qbb, att, rcp, aos, rep):
                q0 = qb * QB
                qa0 = q0 + qbb * 512
                nkc_q = (qa0 + 512) // 128
                ps_o = psO.tile([128, 512], f32, tag="o",
                                name=f"pso{rep}_{b}_{qb}_{qbb}")
                for kc in range(nkc_q):
                    kst = kc * 128
                    lo = max(qa0, kst) - qa0
                    off = kc * 2 * QB
                    vn = vns[kc]
                    for h in range(2):
                        nc.tensor.matmul(
                            ps_o[h * 64:(h + 1) * 64, lo:512],
                            vn[:, h * 64:(h + 1) * 64],
                            att[:, off + h * QB + qbb * 512 + lo:
                                off + h * QB + (qbb + 1) * 512],
                            start=(kc == 0), stop=(kc == nkc_q - 1),
                            tile_position=(0, h * 64),
                            skip_group_check=True,
                        )
                for h in range(2):
                    nc.vector.tensor_tensor(
                        aos[h * 64:(h + 1) * 64, qa0:qa0 + 512],
                        ps_o[h * 64:(h + 1) * 64, :],
                        rcp[h * 64:(h + 1) * 64,
                            h * QB + qbb * 512:h * QB + (qbb + 1) * 512],
                        op=MUL,
                    )

            def outproj_ec(b, ts, ec, aos, rep):
                t0b = b * S
                po = pop.tile([128, len(ts) * TB], f16, tag="po",
                              name=f"po{rep}_{b}_{ec}_{ts[0]}")
                for tt, t in enumerate(ts):
                    ps_p = psA.tile([128, TB], f32, tag="mm512",
                                    name=f"psp{rep}_{b}_{ec}_{t}")
                    nc.tensor.matmul(
                        ps_p[:],
                        wo_sb[:, ec * 128:(ec + 1) * 128],
                        aos[:, t * TB:(t + 1) * TB],
                        start=True, stop=True,
                    )
                    nc.vector.tensor_copy(
                        po[:, tt * TB:(tt + 1) * TB], ps_p[:])
                nc.sync.dma_start(
                    outp.ap()[ec * 128:(ec + 1) * 128,
                              t0b + ts[0] * TB:t0b + (ts[-1] + 1) * TB],
                    po[:],
                )

            for rep in range(repeats):
                # prologue: only batch-0 tokens [0,1024) serially; the rest
                # becomes fill work inside the first scores loop
                tiles = alloc_qkv(0)
                for t in (0, 1):
                    xth = xt00 if (t == 0 and rep == 0) else qkv_dma(0, t, rep)
                    for gi in range(3):
                        qkv_group(0, t, tiles, xth, gi, rep)
                vnat(0, tiles, rep, 0, 8)
                pro_fill = []
                for t in (2, 3):
                    xth = qkv_dma(0, t, rep)
                    for gi in range(3):
                        pro_fill.append(
                            (lambda t=t, xth=xth, gi=gi, tl=tiles:
                             qkv_group(0, t, tl, xth, gi, rep))
                        )
                pro_fill.append(
                    (lambda tl=tiles: vnat(0, tl, rep, 8, S // 128))
                )
                prev = None  # (b, aos) with half-1 outproj still pending
                for b in range(B):
                    nxt = b + 1 if b + 1 < B else None
                    tiles_next = alloc_qkv(nxt) if nxt is not None else None
                    aos = aop.tile([128, S], f16, tag="ao",
                                   name=f"ao{rep}_{b}")
                    for qb in range(NQB):
                        att = atp.tile([128, (8 if qb == 0 else 16) * 2 * QB],
                                       f16, tag=f"att{qb}",
                                       name=f"att{rep}_{b}_{qb}")
                        acc = acp.tile([128, 2 * QB], f16, tag="acc",
                                       name=f"acc{rep}_{b}_{qb}")
                        fill = []
                        if b == 0 and qb == 0:
                            fill.extend(pro_fill)
                        if nxt is not None:
                            tls = [0] if qb == 0 else [1, 2, 3]
                            for t in tls:
                                xth = qkv_dma(nxt, t, rep)
                                for gi in range(3):
                                    fill.append(
                                        (lambda t=t, xth=xth, gi=gi:
                                         qkv_group(nxt, t, tiles_next,
                                                   xth, gi, rep))
                                    )
                        if qb == 0 and prev is not None:
                            pb, paos = prev
                            for ec in range(KC):
                                fill.append(
                                    (lambda ec=ec, pb=pb, paos=paos:
                                     outproj_ec(pb, [2, 3], ec, paos, rep))
                                )
                            prev = None
                        if qb == 1:
                            for ec in range(KC):
                                fill.append(
                                    (lambda ec=ec: outproj_ec(b, [0, 1], ec,
                                                              aos, rep))
                                )
                        scores(b, qb, tiles, att, acc, rep, fill)
                        rcp = denom(b, qb, acc, rep)
                        for qbb in range(QB // 512):
                            attv_qbb(b, qb, qbb, att, rcp, aos, rep)
                            if nxt is None and qb == 1:
                                # last batch: drain the final outproj chunk
                                # as soon as its 512 tokens are normalized
                                for ec in range(KC):
                                    outproj_ec(b, [2 + qbb], ec, aos, rep)
                    if nxt is not None:
                        vnat(nxt, tiles_next, rep)
                    prev = (b, aos)
                    tiles = tiles_next
    nc.compile()
    return nc


_CACHE = {}


def _get_nc(repeats=1):
    if repeats not in _CACHE:
        _CACHE[repeats] = build(repeats)
    return _CACHE[repeats]


def make_in_maps(x, W_qkv, b_qkv, W_out, b_out):
    x = np.asarray(x, dtype=np.float32)
    W_qkv = np.asarray(W_qkv, dtype=np.float32)
    b_qkv = np.asarray(b_qkv, dtype=np.float32)
    W_out = np.asarray(W_out, dtype=np.float32)
    xT = np.ascontiguousarray(x.reshape(TOK, E).T).astype(np.float16)
    trim = np.ascontiguousarray(
        np.triu(np.ones((128, 128), dtype=np.float32))
    ).astype(np.float16)
    in_maps = []
    for c in range(8):
        cs = slice(c * 128, (c + 1) * 128)
        in_maps.append({
            "xT": xT,
            "wq": np.ascontiguousarray(
                W_qkv[:, c * 128:(c + 1) * 128]).astype(np.float16),
            "wk": np.ascontiguousarray(
                W_qkv[:, E + c * 128:E + (c + 1) * 128]).astype(np.float16),
            "wv": np.ascontiguousarray(
                W_qkv[:, 2 * E + c * 128:2 * E + (c + 1) * 128]
            ).astype(np.float16),
            "wo": np.ascontiguousarray(W_out[cs, :]).astype(np.float16),
            "bq": np.ascontiguousarray(b_qkv[c * 128:(c + 1) * 128, None]),
            "bk": np.ascontiguousarray(
                b_qkv[E + c * 128:E + (c + 1) * 128, None]),
            "bv": np.ascontiguousarray(
                b_qkv[2 * E + c * 128:2 * E + (c + 1) * 128, None]),
            "tri": trim,
            "idd": np.eye(128, dtype=np.float32).astype(np.float16),
        })
    return in_maps


def gather(results, b_out):
    total = np.zeros((E, TOK), dtype=np.float64)
    for c in range(8):
        total += results[c]["outp"].astype(np.float64)
    out = total.T.astype(np.float32) + np.asarray(b_out, dtype=np.float32)
    return np.ascontiguousarray(out.reshape(B, S, E)).astype(np.float32)


def kernel(x, W_qkv, b_qkv, W_out, b_out):
    nc = _get_nc(1)
    in_maps = make_in_maps(x, W_qkv, b_qkv, W_out, b_out)
    res = bass_utils.run_bass_kernel_spmd(nc, in_maps, core_ids=list(range(8)))
    return gather(res.results, b_out)
